# revision 15
# baseline (speedup 1.0000x reference)
"""CrossAttention Trainium2 kernel — 8-core tensor-parallel (2 heads/core).

Self-contained: builds a Bass/Tile kernel, shards the full inputs across the
8 NeuronCores, runs via the axon PJRT path, and gathers the full output.
"""

import sys
import time

for _p in ("/opt/trn_rl_repo", "/root/.axon_site/_ro/trn_rl_repo"):
    if _p not in sys.path:
        sys.path.insert(0, _p)

import numpy as np
from contextlib import ExitStack

import concourse.bacc as bacc
import concourse.mybir as mybir
import concourse.tile as tile
from concourse.mybir import ActivationFunctionType as AF
from concourse.mybir import AluOpType as ALU

# ---------------------------------------------------------------- problem dims
D = 1024
H = 16
DH = 64
TQ = 2048
TKV = 2048
B = 2
NCORES = 8
HPC = H // NCORES          # heads per core = 2
MPC = HPC * DH             # dims per core  = 128
T = B * TQ                 # token axis (b-grouped) = 4096
NROT = 32                  # rotated channels per head (frac 0.5 of 64)
NHEADS_ROT = 12            # rotated heads (frac 0.75 of 16)
MAX_WL = 8192.0

F32 = mybir.dt.float32

TWO_PI = 2.0 * np.pi
INV_2PI = float(np.float32(1.0 / TWO_PI))
MAGIC = float(np.float32(1.5 * 2 ** 23))
CW1 = float(np.float32(6.28125))
CW2 = float(np.float32(TWO_PI - 6.28125))
CW3 = float(TWO_PI - CW1 - float(np.float32(TWO_PI - 6.28125)))


# ---------------------------------------------------------------- bass builder
def build_kernel(use_mask: bool, debug: bool = False):
    nc = bacc.Bacc("TRN2", target_bir_lowering=False, debug=False,
                   enable_asserts=True, num_devices=NCORES)

    xqT = nc.dram_tensor("xqT", [D, T], F32, kind="ExternalInput").ap()
    xkvT = nc.dram_tensor("xkvT", [D, T], F32, kind="ExternalInput").ap()
    wqT = nc.dram_tensor("wqT", [D, MPC], F32, kind="ExternalInput").ap()
    wkT = nc.dram_tensor("wkT", [D, MPC], F32, kind="ExternalInput").ap()
    wvT = nc.dram_tensor("wvT", [D, MPC], F32, kind="ExternalInput").ap()
    wqTs = nc.dram_tensor("wqTs", [D, MPC], F32, kind="ExternalInput").ap()
    wkTs = nc.dram_tensor("wkTs", [D, MPC], F32, kind="ExternalInput").ap()
    bq_d = nc.dram_tensor("bq", [MPC, 1], F32, kind="ExternalInput").ap()
    bk_d = nc.dram_tensor("bk", [MPC, 1], F32, kind="ExternalInput").ap()
    bv_d = nc.dram_tensor("bv", [1, MPC], F32, kind="ExternalInput").ap()
    bqs_d = nc.dram_tensor("bqs", [MPC, 1], F32, kind="ExternalInput").ap()
    bks_d = nc.dram_tensor("bks", [MPC, 1], F32, kind="ExternalInput").ap()
    wo0_d = nc.dram_tensor("woT0", [128, D], F32, kind="ExternalInput").ap()
    wo1_d = nc.dram_tensor("woT1", [128, D], F32, kind="ExternalInput").ap()
    invf_d = nc.dram_tensor("invf", [1, 128], F32, kind="ExternalInput").ap()
    posq_d = nc.dram_tensor("posq", [1, T], F32, kind="ExternalInput").ap()
    posk_d = nc.dram_tensor("posk", [1, T], F32, kind="ExternalInput").ap()
    if use_mask:
        mask_d = nc.dram_tensor("maskT", [TKV, B, TQ], F32, kind="ExternalInput").ap()
    outT = nc.dram_tensor("outT", [D, T], F32, kind="ExternalOutput").ap()
    dbg = {}
    if debug:
        for nm, shp in (("d_qT", [MPC, T]), ("d_kT", [MPC, T]),
                        ("d_shq", [MPC, T]), ("d_shk", [MPC, T]),
                        ("d_vn0", [128, B * (TKV // 128) * 128]),
                        ("d_vn1", [128, B * (TKV // 128) * 128]),
                        ("d_ot0", [128, T]), ("d_ot1", [128, T]),
                        ("d_pt", [128, 512]), ("d_rb", [128, 512]),
                        ("d_sums", [1, 512]),
                        ("d_st", [128, 512]), ("d_sin", [128, 512]),
                        ("d_cos", [128, 512])):
            dbg[nm] = nc.dram_tensor(nm, shp, F32, kind="ExternalOutput").ap()

    KC = D // 128            # 8 contraction chunks for projections
    TB = T // 512            # 8 token blocks of 512
    QB = TQ // 512           # 4 query blocks per batch
    KVC = TKV // 128         # 16 kv chunks per batch
    VW = 128                 # v chunk width: [ones, zeros, dims]

    with tile.TileContext(nc) as tc:
        with ExitStack() as octx:
            persist = octx.enter_context(tc.tile_pool(name="persist", bufs=1))

            qT = persist.tile([MPC, T], F32, tag="qT")
            kT = persist.tile([MPC, T], F32, tag="kT")
            vn = [persist.tile([128, B * KVC * VW], F32, tag=f"vn{h}",
                                 name=f"vn{h}") for h in range(HPC)]
            vn3 = [v.rearrange("p (c w) -> p c w", w=VW) for v in vn]
            ot0 = persist.tile([128, T], F32, tag="ot0")
            ot1 = persist.tile([128, T], F32, tag="ot1")
            wo0 = persist.tile([128, D], F32, tag="wo0")
            wo1 = persist.tile([128, D], F32, tag="wo1")
            invf_sb = persist.tile([1, 128], F32, tag="invf")
            bq_sb = persist.tile([MPC, 1], F32, tag="bq")
            bk_sb = persist.tile([MPC, 1], F32, tag="bk")
            bv_sb = persist.tile([1, MPC], F32, tag="bv")
            bqs_sb = persist.tile([MPC, 1], F32, tag="bqs")
            bks_sb = persist.tile([MPC, 1], F32, tag="bks")
            ones_row = persist.tile([1, 128], F32, tag="ones_row")

            nc.sync.dma_start(wo0[:], wo0_d[:])
            nc.sync.dma_start(wo1[:], wo1_d[:])
            nc.sync.dma_start(invf_sb[:], invf_d[:])
            nc.sync.dma_start(bq_sb[:], bq_d[:])
            nc.sync.dma_start(bk_sb[:], bk_d[:])
            nc.sync.dma_start(bv_sb[:], bv_d[:])
            nc.sync.dma_start(bqs_sb[:], bqs_d[:])
            nc.sync.dma_start(bks_sb[:], bks_d[:])
            nc.vector.memset(ones_row[:], 1.0)
            nc.vector.memset(ot0[0:64, :], 0.0)
            nc.vector.memset(ot0[0:1, :], 1.0)            # ones row for bo
            nc.vector.memset(ot1[0:64, :], 0.0)
            for h in range(HPC):
                nc.vector.memset(vn3[h][:, :, 0:1], 1.0)  # ones cols for sums
                nc.vector.memset(vn3[h][:, :, 1:DH], 0.0)

            wq_sb, wk_sb, wv_sb, wqs_sb, wks_sb = [], [], [], [], []
            for kc in range(KC):
                for lst, src, tg in ((wq_sb, wqT, "wq"), (wk_sb, wkT, "wk"),
                                     (wv_sb, wvT, "wv"), (wqs_sb, wqTs, "wqs"),
                                     (wks_sb, wkTs, "wks")):
                    t = persist.tile([128, MPC], F32, tag=f"{tg}{kc}",
                                     name=f"{tg}{kc}")
                    nc.sync.dma_start(t[:], src[kc * 128:(kc + 1) * 128, :])
                    lst.append(t)

            # ---------------- phases 1+1.5 share the swapped projections ---
            midctx = ExitStack()
            mid = midctx.enter_context(tc.tile_pool(name="mid", bufs=1))
            shq = mid.tile([MPC, T], F32, tag="shq")
            shk = mid.tile([MPC, T], F32, tag="shk")

            # ---------------- phase 1: q/k/v projections -------------------
            with ExitStack() as ctx:
                xpool = ctx.enter_context(tc.tile_pool(name="xio", bufs=8))
                pjq = ctx.enter_context(tc.tile_pool(name="pjq", bufs=2, space="PSUM"))
                pjk = ctx.enter_context(tc.tile_pool(name="pjk", bufs=2, space="PSUM"))
                pjv = ctx.enter_context(tc.tile_pool(name="pjv", bufs=1, space="PSUM"))
                pjqs = ctx.enter_context(tc.tile_pool(name="pjqs", bufs=1, space="PSUM"))
                pjks = ctx.enter_context(tc.tile_pool(name="pjks", bufs=1, space="PSUM"))

                for tb in range(TB):
                    q_ps = pjq.tile([MPC, 512], F32, tag="q_ps")
                    k_ps = pjk.tile([MPC, 512], F32, tag="k_ps")
                    v_ps = pjv.tile([128, 512], F32, tag="v_ps")
                    qs_ps = pjqs.tile([MPC, 512], F32, tag="qs_ps")
                    ks_ps = pjks.tile([MPC, 512], F32, tag="ks_ps")
                    xq_ts, xkv_ts = [], []
                    for kc in range(KC):
                        xq_t = xpool.tile([128, 512], F32, tag="xq")
                        nc.sync.dma_start(
                            xq_t[:], xqT[kc * 128:(kc + 1) * 128,
                                         tb * 512:(tb + 1) * 512])
                        xkv_t = xpool.tile([128, 512], F32, tag="xkv")
                        nc.sync.dma_start(
                            xkv_t[:], xkvT[kc * 128:(kc + 1) * 128,
                                           tb * 512:(tb + 1) * 512])
                        xq_ts.append(xq_t)
                        xkv_ts.append(xkv_t)
                        st = kc == 0
                        sp = kc == KC - 1
                        nc.tensor.matmul(q_ps[:], wq_sb[kc][:], xq_ts[kc][:],
                                         start=st, stop=sp)
                        nc.tensor.matmul(k_ps[:], wk_sb[kc][:], xkv_ts[kc][:],
                                         start=st, stop=sp)
                        nc.tensor.matmul(qs_ps[:], wqs_sb[kc][:], xq_ts[kc][:],
                                         start=st, stop=sp)
                        nc.tensor.matmul(ks_ps[:], wks_sb[kc][:], xkv_ts[kc][:],
                                         start=st, stop=sp)
                    # v natural layout: one sequential accumulation group per
                    # 128-token column slice (interleaved groups in one PSUM
                    # bank are rejected); bias added via K=1 ones x bv matmul.
                    for tc4 in range(4):
                        for kc in range(KC):
                            nc.tensor.matmul(
                                v_ps[:, tc4 * 128:(tc4 + 1) * 128],
                                xkv_ts[kc][:, tc4 * 128:(tc4 + 1) * 128],
                                wv_sb[kc][:], start=(kc == 0), stop=False)
                        nc.tensor.matmul(v_ps[:, tc4 * 128:(tc4 + 1) * 128],
                                         ones_row[:], bv_sb[:],
                                         start=False, stop=True)
                    nc.scalar.activation(qT[:, tb * 512:(tb + 1) * 512], q_ps[:],
                                         AF.Identity, bias=bq_sb[:])
                    nc.scalar.activation(kT[:, tb * 512:(tb + 1) * 512], k_ps[:],
                                         AF.Identity, bias=bk_sb[:])
                    nc.scalar.activation(shq[:, tb * 512:(tb + 1) * 512],
                                         qs_ps[:], AF.Identity, bias=bqs_sb[:])
                    nc.scalar.activation(shk[:, tb * 512:(tb + 1) * 512],
                                         ks_ps[:], AF.Identity, bias=bks_sb[:])
                    # scatter v chunks into per-head stationary tensors
                    glob = tb * 4
                    for tc4 in range(4):
                        ch = glob + tc4          # global 128-token chunk index
                        for h in range(HPC):
                            nc.scalar.activation(
                                vn[h][:, ch * VW + DH:ch * VW + 2 * DH],
                                v_ps[:, tc4 * 128 + h * DH:
                                     tc4 * 128 + (h + 1) * DH],
                                AF.Identity)

            # ---------------- phase 1.5: rotary on qT / kT -----------------
            # Tables are full-height [128, 512] with rows aligned to the x
            # rows they rotate (rows 0-31 head0, 64-95 head1; rest unused) so
            # every DVE op sees partition-aligned operands.
            with ExitStack() as ctx:
                rot = ctx.enter_context(tc.tile_pool(name="rot", bufs=2))
                radp = ctx.enter_context(tc.tile_pool(name="radp", bufs=2,
                                                      space="PSUM"))
                for src, xt, shx in ((0, qT, shq), (1, kT, shk)):
                    pos_d = posq_d if src == 0 else posk_d
                    for blk in range(TB):
                        sl = slice(blk * 512, (blk + 1) * 512)
                        pos_t = rot.tile([1, 512], F32, tag="pos")
                        nc.sync.dma_start(pos_t[:], pos_d[:, sl])
                        rad = radp.tile([128, 512], F32, tag="rad")
                        nc.tensor.matmul(rad[:], invf_sb[:], pos_t[:],
                                         start=True, stop=True)
                        kr = rot.tile([128, 512], F32, tag="kr")
                        nc.vector.tensor_scalar(kr[:], rad[:], INV_2PI, MAGIC,
                                                ALU.mult, ALU.add)
                        nc.vector.tensor_scalar_sub(kr[:], kr[:], MAGIC)
                        radm = rot.tile([128, 512], F32, tag="radm")
                        nc.vector.cody_waite_cascade(radm[:], rad[:], kr[:],
                                                     CW1, CW2, CW3)
                        wrap0 = rot.tile([128, 512], F32, tag="wrap0")
                        nc.vector.add_range_wrap(wrap0[:], radm[:], 0.0,
                                                 float(np.pi), float(TWO_PI))
                        sin_t = rot.tile([128, 512], F32, tag="sin")
                        nc.scalar.activation(sin_t[:], wrap0[:], AF.Sin)
                        wrap = rot.tile([128, 512], F32, tag="wrap")
                        nc.vector.add_range_wrap(wrap[:], radm[:],
                                                 float(np.pi / 2),
                                                 float(np.pi), float(TWO_PI))
                        cos_t = rot.tile([128, 512], F32, tag="cos")
                        nc.scalar.activation(cos_t[:], wrap[:], AF.Sin)
                        if debug and src == 0 and blk == 0:
                            nc.sync.dma_start(dbg["d_sin"][:], sin_t[:])
                            nc.sync.dma_start(dbg["d_cos"][:], cos_t[:])

                        t1 = rot.tile([128, 512], F32, tag="t1")
                        t2 = rot.tile([128, 512], F32, tag="t2")
                        for h in range(HPC):
                            r = slice(h * DH, h * DH + NROT)
                            nc.vector.tensor_mul(t1[r, :], xt[r, sl],
                                                 cos_t[r, :])
                            nc.vector.tensor_mul(t2[r, :], shx[r, sl],
                                                 sin_t[r, :])
                            nc.vector.tensor_add(xt[r, sl], t1[r, :],
                                                 t2[r, :])

            if debug:
                nc.sync.dma_start(dbg["d_shq"][:], shq[:])
                nc.sync.dma_start(dbg["d_shk"][:], shk[:])
            midctx.close()

            # ---------------- phase 2: attention per (b, qb, h) ------------
            with ExitStack() as ctx:
                stp = ctx.enter_context(tc.tile_pool(name="stp", bufs=4,
                                                     space="PSUM"))
                otp = ctx.enter_context(tc.tile_pool(name="otp", bufs=2,
                                                     space="PSUM"))
                ptp = ctx.enter_context(tc.tile_pool(name="ptp", bufs=18))
                sml = ctx.enter_context(tc.tile_pool(name="sml", bufs=2))
                if use_mask:
                    mkp = ctx.enter_context(tc.tile_pool(name="mkp", bufs=17))

                for b in range(B):
                    for qb in range(QB):
                        qsl = slice(b * TQ + qb * 512, b * TQ + (qb + 1) * 512)
                        mtiles = []
                        if use_mask:
                            for kc in range(KVC):
                                mt = mkp.tile([128, 512], F32, tag="mk")
                                nc.sync.dma_start(
                                    mt[:], mask_d[kc * 128:(kc + 1) * 128, b,
                                                  qb * 512:(qb + 1) * 512])
                                mtiles.append(mt)
                        for h in range(HPC):
                            hs = slice(h * DH, (h + 1) * DH)
                            pts = []
                            for kc in range(KVC):
                                st_ps = stp.tile([128, 512], F32, tag="st")
                                nc.tensor.matmul(
                                    st_ps[:],
                                    kT[hs, b * TKV + kc * 128:
                                       b * TKV + (kc + 1) * 128],
                                    qT[hs, qsl], start=True, stop=True)
                                pt = ptp.tile([128, 512], F32, tag="pt")
                                if debug and b == 0 and qb == 0 and h == 0 and kc == 0:
                                    st_sb = ptp.tile([128, 512], F32, tag="st_sb", bufs=1)
                                    nc.vector.tensor_copy(st_sb[:], st_ps[:])
                                    nc.sync.dma_start(dbg["d_st"][:], st_sb[:])
                                nc.scalar.activation(pt[:], st_ps[:], AF.Exp)
                                if use_mask:
                                    nc.vector.tensor_mul(pt[:], pt[:],
                                                          mtiles[kc][:])
                                if debug and b == 0 and qb == 0 and h == 0 and kc == 0:
                                    nc.sync.dma_start(dbg["d_pt"][:], pt[:])
                                pts.append(pt)
                            ot_ps = otp.tile([128, 512], F32, tag="ot")
                            for kc in range(KVC):
                                cw = (b * KVC + kc) * VW
                                nc.tensor.matmul(ot_ps[:],
                                                 vn[h][:, cw:cw + VW],
                                                 pts[kc][:],
                                                 start=(kc == 0),
                                                 stop=(kc == KVC - 1))
                            recip = sml.tile([1, 512], F32, tag="recip")
                            nc.vector.reciprocal_approx_fast(
                                out=recip[:], in_=ot_ps[0:1, :])
                            rb = sml.tile([128, 512], F32, tag="rb")
                            nc.gpsimd.partition_broadcast(rb[:], recip[:])
                            if debug and b == 0 and qb == 0 and h == 0:
                                nc.sync.dma_start(dbg["d_rb"][:], rb[:])
                                nc.sync.dma_start(dbg["d_sums"][:],
                                                  recip[:])
                            dst = ot0 if h == 0 else ot1
                            nc.vector.tensor_mul(dst[DH:2 * DH, qsl],
                                                 ot_ps[DH:2 * DH, :],
                                                 rb[DH:2 * DH, :])

            if debug:
                nc.sync.dma_start(dbg["d_qT"][:], qT[:])
                nc.sync.dma_start(dbg["d_kT"][:], kT[:])
                nc.sync.dma_start(dbg["d_vn0"][:], vn[0][:])
                nc.sync.dma_start(dbg["d_vn1"][:], vn[1][:])
                nc.sync.dma_start(dbg["d_ot0"][:], ot0[:])
                nc.sync.dma_start(dbg["d_ot1"][:], ot1[:])

            # ---------------- phase 3: output projection -------------------
            with ExitStack() as ctx:
                outp = ctx.enter_context(tc.tile_pool(name="outp", bufs=4,
                                                      space="PSUM"))
                osb = ctx.enter_context(tc.tile_pool(name="osb", bufs=4))
                for jc in range(KC):
                    for tb in range(TB):
                        o_ps = outp.tile([128, 512], F32, tag="o")
                        nc.tensor.matmul(o_ps[:],
                                         wo0[:, jc * 128:(jc + 1) * 128],
                                         ot0[:, tb * 512:(tb + 1) * 512],
                                         start=True, stop=False)
                        nc.tensor.matmul(o_ps[:],
                                         wo1[:, jc * 128:(jc + 1) * 128],
                                         ot1[:, tb * 512:(tb + 1) * 512],
                                         start=False, stop=True)
                        o_sb = osb.tile([128, 512], F32, tag="o_sb")
                        nc.scalar.activation(o_sb[:], o_ps[:], AF.Identity)
                        nc.sync.dma_start(
                            outT[jc * 128:(jc + 1) * 128,
                                 tb * 512:(tb + 1) * 512], o_sb[:])

    nc.compile()
    return nc


# ---------------------------------------------------------------- pjrt runner
def _make_runner(nc, n_cores=NCORES):
    import jax
    from jax.sharding import Mesh, PartitionSpec
    from jax.experimental.shard_map import shard_map
    from concourse.bass2jax import (_bass_exec_p, install_neuronx_cc_hook,
                                    partition_id_tensor)

    install_neuronx_cc_hook()
    partition_name = (nc.partition_id_tensor.name
                      if nc.partition_id_tensor else None)
    in_names, out_names, out_avals, zero_shapes = [], [], [], []
    for alloc in nc.m.functions[0].allocations:
        if not isinstance(alloc, mybir.MemoryLocationSet):
            continue
        name = alloc.memorylocations[0].name
        if alloc.kind == "ExternalInput":
            if name != partition_name:
                in_names.append(name)
        elif alloc.kind == "ExternalOutput":
            shape = tuple(alloc.tensor_shape)
            dtype = mybir.dt.np(alloc.dtype)
            out_names.append(name)
            out_avals.append(jax.core.ShapedArray(shape, dtype))
            zero_shapes.append((shape, dtype))
    n_params = len(in_names)
    n_outs = len(out_avals)
    all_in_names = list(in_names) + list(out_names)
    if partition_name is not None:
        all_in_names.append(partition_name)

    def _body(*args):
        operands = list(args)
        if partition_name is not None:
            operands.append(partition_id_tensor())
        return tuple(_bass_exec_p.bind(
            *operands, out_avals=tuple(out_avals), in_names=tuple(all_in_names),
            out_names=tuple(out_names), lowering_input_output_aliases=(),
            sim_require_finite=True, sim_require_nnan=True, nc=nc))

    devices = jax.devices()[:n_cores]
    mesh = Mesh(np.asarray(devices), ("core",))
    in_specs = (PartitionSpec("core"),) * (n_params + n_outs)
    out_specs = (PartitionSpec("core"),) * len(out_names)
    donate = tuple(range(n_params, n_params + n_outs))
    sharded = jax.jit(
        shard_map(_body, mesh=mesh, in_specs=in_specs, out_specs=out_specs,
                  check_rep=False),
        donate_argnums=donate, keep_unused=True)

    def run(in_maps, time_iters=0):
        per_core = [[np.asarray(m[name]) for name in in_names]
                    for m in in_maps]
        concat_in = [np.concatenate([per_core[c][i] for c in range(n_cores)],
                                    axis=0) for i in range(n_params)]

        def zeros():
            return [np.zeros((n_cores * s[0], *s[1:]), d)
                    for s, d in zero_shapes]

        import jax
        out_arrs = sharded(*concat_in, *zeros())
        jax.block_until_ready(out_arrs)
        times = []
        for _ in range(time_iters):
            t0 = time.perf_counter()
            o = sharded(*concat_in, *zeros())
            jax.block_until_ready(o)
            times.append(time.perf_counter() - t0)
            out_arrs = o
        results = [
            {name: np.asarray(out_arrs[i]).reshape(n_cores,
                                                   *out_avals[i].shape)[c]
             for i, name in enumerate(out_names)}
            for c in range(n_cores)]
        return results, times

    return run


# ---------------------------------------------------------------- host shard
def _inv_freq_signed():
    nb = NROT // 2  # 16 distinct frequencies
    freq = MAX_WL ** (2.0 / NROT * np.linspace(0.0, float(nb), nb))
    inv = (1.0 / freq).astype(np.float32)
    s = np.zeros(128, np.float32)
    for h in range(HPC):
        s[h * DH:h * DH + NROT:2] = -inv
        s[h * DH + 1:h * DH + NROT:2] = inv
    return s


def make_in_maps(inputs_q, inputs_kv, mask, q_positions, kv_positions,
                 Wq, bq, Wk, bk, Wv, bv, Wo, bo, use_mask):
    f32 = np.float32
    xqT = np.ascontiguousarray(
        np.asarray(inputs_q, f32).transpose(2, 1, 0).reshape(D, T))
    xkvT = np.ascontiguousarray(
        np.asarray(inputs_kv, f32).transpose(2, 1, 0).reshape(D, T))
    posq = np.ascontiguousarray(
        np.asarray(q_positions, f32).T.reshape(1, T))
    posk = np.ascontiguousarray(
        np.asarray(kv_positions, f32).T.reshape(1, T))
    scale = f32(1.0 / np.sqrt(DH))
    perm = np.arange(MPC)
    for h in range(HPC):
        base = h * DH
        perm[base:base + NROT:2] = np.arange(base + 1, base + NROT + 1, 2)
        perm[base + 1:base + NROT:2] = np.arange(base, base + NROT, 2)
    Wq, Wk, Wv, Wo = (np.asarray(a, f32) for a in (Wq, Wk, Wv, Wo))
    bq, bk, bv, bo = (np.asarray(a, f32) for a in (bq, bk, bv, bo))
    invf = _inv_freq_signed()
    if use_mask:
        maskT = np.ascontiguousarray((np.asarray(mask) > 0).astype(f32))

    in_maps = []
    for c in range(NCORES):
        sl = slice(c * MPC, (c + 1) * MPC)
        m = {
            "xqT": xqT, "xkvT": xkvT, "posq": posq, "posk": posk,
            "wqT": np.ascontiguousarray((scale * Wq[sl, :]).T),
            "wkT": np.ascontiguousarray(Wk[sl, :].T),
            "wvT": np.ascontiguousarray(Wv[sl, :].T),
            "wqTs": np.ascontiguousarray((scale * Wq[sl, :][perm, :]).T),
            "wkTs": np.ascontiguousarray(Wk[sl, :][perm, :].T),
            "bq": (scale * bq[sl]).reshape(MPC, 1),
            "bk": bk[sl].reshape(MPC, 1).copy(),
            "bv": bv[sl].reshape(1, MPC).copy(),
            "bqs": (scale * bq[sl])[perm].reshape(MPC, 1).copy(),
            "bks": bk[sl][perm].reshape(MPC, 1).copy(),
            "woT0": np.ascontiguousarray(np.concatenate(
                [(bo if c == 0 else np.zeros_like(bo))[None, :],
                 np.zeros((DH - 1, D), f32),
                 Wo[:, c * MPC:c * MPC + DH].T], axis=0)),
            "woT1": np.ascontiguousarray(np.concatenate(
                [np.zeros((DH, D), f32),
                 Wo[:, c * MPC + DH:(c + 1) * MPC].T], axis=0)),
            "invf": (invf if (c + 1) * HPC <= NHEADS_ROT
                     else np.zeros_like(invf)).reshape(1, 128),
        }
        if use_mask:
            m["maskT"] = maskT
        in_maps.append(m)
    return in_maps


_CACHE = {}


def _get(use_mask):
    if use_mask not in _CACHE:
        nc = build_kernel(use_mask)
        _CACHE[use_mask] = (nc, _make_runner(nc))
    return _CACHE[use_mask]


def kernel(inputs_q, inputs_kv, mask, q_positions, kv_positions,
           Wq, bq, Wk, bk, Wv, bv, Wo, bo, _time_iters=0):
    use_mask = not bool(np.all(np.asarray(mask) > 0))
    nc, run = _get(use_mask)
    in_maps = make_in_maps(inputs_q, inputs_kv, mask, q_positions,
                           kv_positions, Wq, bq, Wk, bk, Wv, bv, Wo, bo,
                           use_mask)
    results, times = run(in_maps, time_iters=_time_iters)
    acc = np.zeros((D, T), np.float64)
    for c in range(NCORES):
        acc += results[c]["outT"]
    out = acc.astype(np.float32).reshape(D, B, TQ).transpose(2, 1, 0)
    out = np.ascontiguousarray(out)
    if _time_iters:
        kernel._last_times = times
    return out
